# revision 69
# baseline (speedup 1.0000x reference)
"""Trainium2 Bass kernel for nn_CrossAttention_5385888989393.

Contract: kernel(**inputs) takes FULL inputs (batch 8) and returns the FULL
output, sharding batch-parallel across 8 NeuronCores (1 batch element per
core, no collectives).

Algorithm per batch (channel attention, contraction over spatial n=4096):
    G     = f_m @ f_n^T                     [512, 512]  Gram over n
    T2T   = G^T @ Wq^T                      [512, 512]  (G stationary)
    D^T_h = Wk_h-contraction with T2T       [64, 64] per head (diag tiles)
    E^T   = exp(D^T * scale) * headmask     (softmax numerator, transposed)
    SE_h  = E_h @ Wv_h   (via lhsT = E^T)   [64, 512]
    S_h   = SE_h / rowsum(E_h)              (deferred softmax normalization)
    M^T   = S-contraction with Wout^T       [512, 512]
    out   = (M @ f_n) + bout                [512, 4096]

This is ~2x fewer FLOPs than the naive q/k/v-projection path because the
spatial dimension collapses through the Gram matrix immediately.

v3 (~96.7us HW, vs 131.3us f32r baseline): the data path runs in bfloat16
(rel err ~6e-3, well under the 2e-2 gate; bf16 matmuls run 1 cyc/row at
any width and transposes 1 cyc/row vs 1.5 for f32r, and DMA bytes halve).
Key structural points:
 - f_m arrives host-transposed (fmT) so it needs no device transposes;
   f_n is transposed on-PE per 128-row subchunk (it must also stay
   resident in natural layout as the phase-3 rhs, so an XBAR DMA
   transpose would double its DMA traffic and lose).
 - All inputs are SBUF-resident; every input DMA is issued upfront on the
   SP queue in exact consumption order (fn0, fm0, fn1, ...), one fused
   dma_start per 1MB chunk (each dma_start costs ~620ns sequencer + ~650ns
   DGE + 900ns completion-semaphore).  Chunk 0 is split per-subchunk
   across SP+ACT so the first Gram matmul starts ~4us in.
 - Phase 1 is software-pipelined one unit ahead (transposes for unit u+1
   run before the Gram matmuls of unit u) so the psum->sbuf copy latency
   (DVE/ACT alternating) hides under Gram matmuls; PE is gap-free.
 - PE warm-up multiplies a memset-zero tile so the clock ramps without
   waiting on DMA; warm-up results fold into the identity/dmask tiles to
   stay live.  The D-block matmuls run bt-outer so they interleave with
   T2T casts; rowsum/reciprocal/SE/scale are fused per head-pair tile.
 - Outputs are staged in ot-pairs with one fused DMA per pair; the final
   chunk uses per-tile DMAs to shorten the drain tail.
"""
import sys

if "/opt/trn_rl_repo" not in sys.path:
    sys.path.insert(0, "/opt/trn_rl_repo")

import numpy as np
import ml_dtypes

import concourse.bass as bass
import concourse.tile as tile
from concourse import bacc, mybir
from concourse.bass_utils import run_bass_kernel_spmd
F32 = mybir.dt.float32
BF16 = mybir.dt.bfloat16
EXP = mybir.ActivationFunctionType.Exp
CP = mybir.ActivationFunctionType.Copy
IDENT_FN = mybir.ActivationFunctionType.Identity

P = 128          # partitions
C = 512          # channels
CT = C // P      # 4 channel tiles
NN = 4096        # spatial (64*64)
NCH = NN // 512  # 8 column chunks of 512
NSUB = NN // P   # 32 column subchunks of 128
DH = 64
SCALE = DH ** -0.5
B = 8            # batch == n_cores

_CACHED_NC = None
_CACHED_RUNNER = None

_BF = ml_dtypes.bfloat16
_IDENT = np.eye(P, dtype=_BF)
_DMASK = np.kron(np.eye(2, dtype=np.float32), np.ones((DH, DH), np.float32))
_ONES2 = np.ones((P, 2), dtype=_BF)


def _build():
    nc = bacc.Bacc("TRN2", target_bir_lowering=False, debug=False, num_devices=B)

    fmt_d = nc.dram_tensor("fmT", [NN, C], BF16, kind="ExternalInput").ap()
    fn_d = nc.dram_tensor("fn", [C, NN], BF16, kind="ExternalInput").ap()
    wqt_d = nc.dram_tensor("WqT", [C, C], BF16, kind="ExternalInput").ap()
    wkt_d = nc.dram_tensor("WkT", [C, C], BF16, kind="ExternalInput").ap()
    wv_d = nc.dram_tensor("Wv", [C, C], BF16, kind="ExternalInput").ap()
    woutt_d = nc.dram_tensor("WoutT", [C, C], BF16, kind="ExternalInput").ap()
    bout_d = nc.dram_tensor("bout", [C], F32, kind="ExternalInput").ap()
    ident_d = nc.dram_tensor("ident", [P, P], BF16, kind="ExternalInput").ap()
    dmask_d = nc.dram_tensor("dmask", [P, P], F32, kind="ExternalInput").ap()
    ones2_d = nc.dram_tensor("ones2", [P, 2], BF16, kind="ExternalInput").ap()
    out_d = nc.dram_tensor("out", [C, NN], F32, kind="ExternalOutput").ap()

    # DRAM views for fused chunk DMAs
    fmt_v = fmt_d.rearrange("(ch su p) c -> ch p su c", p=P, su=4)
    fn_v = fn_d.rearrange("(ct p) n -> p ct n", p=P)
    out_v = out_d.rearrange("(ot p) n -> p ot n", p=P)
    bout_v = bout_d.rearrange("(t p) -> p t", p=P)

    with tile.TileContext(nc) as tc:
        with (
            tc.tile_pool(name="const", bufs=1) as const,
            tc.tile_pool(name="w", bufs=1) as wpool,
            tc.tile_pool(name="fmst", bufs=1) as fmst,
            tc.tile_pool(name="fnres", bufs=1) as fnres,
            tc.tile_pool(name="ft", bufs=4) as ftpool,
            tc.tile_pool(name="small", bufs=1) as small,
            tc.tile_pool(name="outst", bufs=3) as outst,
            tc.tile_pool(name="gacc", bufs=1, space="PSUM") as gacc,
            tc.tile_pool(name="work", bufs=2, space="PSUM") as work,
        ):
            # ---------- warm-up: ramp the PE clock without waiting on DMA.
            # Multiplying a memset-zero tile gives ~2us of back-to-back PE
            # work starting at t~0; the result (zeros) is folded into the
            # identity tile to keep the chain live.
            zwarm = const.tile([P, P], BF16, tag="zwarm")
            nc.vector.memset(zwarm[:], 0.0)
            # matmul #0 gets its own psum slice so the identity chain (wz ->
            # identu) only waits for it, not the whole warm-up; matmuls 1-13
            # form one accumulation group kept live through the dmask chain.
            warm_ps = work.tile([P, C], F32, tag="wk1", name="warmps")
            nc.tensor.matmul(warm_ps[:, 0:P], zwarm[:], zwarm[:],
                             start=True, stop=True)
            for i in range(1, 22):
                nc.tensor.matmul(warm_ps[:, P:2 * P], zwarm[:], zwarm[:],
                                 start=(i == 1), stop=(i == 21))
            wz = const.tile([P, P], BF16, tag="wz")
            nc.vector.tensor_copy(wz[:], warm_ps[:, 0:P])

            # constants go on the ACT DMA queue so they don't delay the SP
            # queue, which streams fmT/fn in exact consumption order
            ident0 = const.tile([P, P], BF16, tag="ident0")
            nc.scalar.dma_start(ident0[:], ident_d)
            identu = const.tile([P, P], BF16, tag="identu")
            nc.vector.tensor_add(identu[:], ident0[:], wz[:])

            # ---------- phase 1 loads: everything is SBUF-resident, so all
            # input DMAs are issued upfront on SP in exact consumption order
            # (the DMA engines drain the queue in issue order).  Chunk 0 is
            # split per-subchunk so the first Gram matmul starts ~3.5us in.
            g_ps = [
                gacc.tile([P, C], F32, tag=f"g{at}", name=f"g_ps{at}")
                for at in range(CT)
            ]

            fm_ch = [None] * NCH
            fn_ch = [None] * NCH
            for ch in range(NCH):
                fmc = fmst.tile([P, 4 * C], BF16, tag=f"fm{ch}",
                                name=f"fm{ch}")
                fnc = fnres.tile([P, 4 * 512], BF16, tag=f"fn{ch}",
                                 name=f"fn{ch}")
                fm_ch[ch], fn_ch[ch] = fmc, fnc
                if ch == 0:
                    # chunk 0's pieces are spread across three issuing
                    # engines so their DGE-config costs run in parallel
                    nc.sync.dma_start(
                        fnc[:].rearrange("p (ct n) -> p ct n", ct=4)
                        [:, :, 0:P],
                        fn_v[:, :, 0:P],
                    )
                    nc.sync.dma_start(fmc[:, 0:C], fmt_d[0:P, :])
                    nc.scalar.dma_start(
                        fnc[:].rearrange("p (ct n) -> p ct n", ct=4)
                        [:, :, P:2 * P],
                        fn_v[:, :, P:2 * P],
                    )
                    nc.scalar.dma_start(fmc[:, C:2 * C], fmt_d[P:2 * P, :])
                    nc.sync.dma_start(
                        fnc[:].rearrange("p (ct n) -> p ct n", ct=4)
                        [:, :, 2 * P:512],
                        fn_v[:, :, 2 * P:512],
                    )
                    nc.sync.dma_start(
                        fmc[:].rearrange("p (su c) -> p su c", su=4)[:, 2:4],
                        fmt_v[ch][:, 2:4],
                    )
                else:
                    nc.sync.dma_start(
                        fmc[:].rearrange("p (su c) -> p su c", su=4),
                        fmt_v[ch],
                    )
                    nc.sync.dma_start(
                        fnc[:].rearrange("p (ct n) -> p ct n", ct=4),
                        fn_v[:, :, ch * 512:(ch + 1) * 512],
                    )
                # weights interleave into the stream once the pipeline has
                # slack; WqT/WkT are needed first (T2T/D), Wv/WoutT later
                if ch == 3:
                    wq_sb = wpool.tile([P, 4 * C], BF16, tag="wqT")
                    nc.sync.dma_start(
                        wq_sb[:].rearrange("p (rt c) -> p rt c", rt=4),
                        wqt_d.rearrange("(rt p) c -> p rt c", p=P),
                    )
                    wk_sb = wpool.tile([P, 4 * C], BF16, tag="wkT")
                    nc.sync.dma_start(
                        wk_sb[:].rearrange("p (rt c) -> p rt c", rt=4),
                        wkt_d.rearrange("(rt p) c -> p rt c", p=P),
                    )
            # remaining constants ride the ACT queue behind chunk 0's pieces
            # (all are phase-2+ consumers); wz2 keeps warm-up matmuls 1-13
            # live through the dmask chain
            wz2 = const.tile([P, P], F32, tag="wz2")
            nc.vector.tensor_copy(wz2[:], warm_ps[:, P:2 * P])
            dmask0 = const.tile([P, P], F32, tag="dmask0")
            nc.scalar.dma_start(dmask0[:], dmask_d)
            dmask = const.tile([P, P], F32, tag="dmask")
            nc.vector.tensor_add(dmask[:], dmask0[:], wz2[:])

            ones2 = const.tile([P, 2], BF16, tag="ones2")
            nc.scalar.dma_start(ones2[:], ones2_d)
            bout_sb = const.tile([P, CT], F32, tag="bout")
            nc.scalar.dma_start(bout_sb[:], bout_v)

            # Wv/WoutT are not needed until mid-phase-2; loading them after
            # all chunk loads keeps the chunk stream unbroken
            wv_sb = wpool.tile([P, 4 * C], BF16, tag="wv")
            nc.sync.dma_start(
                wv_sb[:].rearrange("p (rt c) -> p rt c", rt=4),
                wv_d.rearrange("(rt p) c -> p rt c", p=P),
            )
            wo_sb = wpool.tile([P, 4 * C], BF16, tag="woT")
            nc.sync.dma_start(
                wo_sb[:].rearrange("p (rt c) -> p rt c", rt=4),
                woutt_d.rearrange("(rt p) c -> p rt c", p=P),
            )

            # ---------- phase 1 compute ------------------------------------
            # Work units are (chunk, su-list): chunk 0 runs per-subchunk so
            # the very first Gram matmul starts as soon as 256KB have landed;
            # later chunks run su-pairs (8 transposes into one 2KB PSUM
            # buffer, one psum->sbuf copy per pair).  From unit 2 on, the
            # loop is software-pipelined one unit ahead (transposes for unit
            # u+1 are emitted before the Gram matmuls of unit u) so the
            # psum->sbuf copy latency hides under Gram matmuls.
            units = [(0, [0]), (0, [1]), (0, [2]), (0, [3])] + [
                (ch, [2 * sp_, 2 * sp_ + 1])
                for ch in range(1, NCH)
                for sp_ in range(2)
            ]
            NU = len(units)

            def emit_tr(u):
                ch, sus = units[u]
                fnc = fn_ch[ch]
                tp = work.tile([P, len(sus) * C], BF16, tag="wk0",
                               name=f"tp{u}")
                for h, su in enumerate(sus):
                    for ct in range(CT):
                        nc.tensor.transpose(
                            tp[:, h * C + ct * P: h * C + (ct + 1) * P],
                            fnc[:, ct * 512 + su * P:
                                ct * 512 + (su + 1) * P],
                            identu[:],
                        )
                fnT = ftpool.tile([P, len(sus) * C], BF16, tag="fnT",
                                  name=f"fnT{u}")
                if u % 2 == 0:
                    nc.vector.tensor_copy(fnT[:], tp[:])
                else:
                    nc.scalar.activation(fnT[:], tp[:], CP)
                return [fnT[:, h * C:(h + 1) * C] for h in range(len(sus))]

            # G copies are interleaved with the final pair's Gram matmuls:
            # each g_ps[at] copy is emitted right after its stop matmul
            G_sb = [
                small.tile([P, C], BF16, tag=f"G{at}", name=f"G{at}")
                for at in range(CT)
            ]

            def emit_gram(u, fnT):
                ch, sus = units[u]
                fmc = fm_ch[ch]
                for h, su in enumerate(sus):
                    s = ch * 4 + su
                    for at in range(CT):
                        nc.tensor.matmul(
                            g_ps[at][:],
                            fmc[:, su * C + at * P: su * C + (at + 1) * P],
                            fnT[h],
                            start=(s == 0),
                            stop=(s == NSUB - 1),
                        )
                        if s == NSUB - 1:
                            if at % 2 == 0:
                                nc.vector.tensor_copy(
                                    G_sb[at][:], g_ps[at][:]
                                )
                            else:
                                nc.scalar.activation(
                                    G_sb[at][:], g_ps[at][:], CP
                                )

            # unit 0: no lookahead (the pipeline is DMA-latency-bound at the
            # start; gram 0 must not queue behind tr 1 on PE)
            fnT0 = emit_tr(0)
            emit_gram(0, fnT0)
            fnT_cur = emit_tr(1)
            for u in range(1, NU):
                fnT_next = emit_tr(u + 1) if u + 1 < NU else None
                emit_gram(u, fnT_cur)
                fnT_cur = fnT_next

            # ---------- phase 2: logits, softmax, value mixing ------------

            # T2T[b, (h,i)] = sum_a G[a, b] * WqT[a, (h,i)]
            T2T_sb = []
            for bt in range(CT):
                ps = work.tile([P, C], F32, tag="wk1", name="t2tps")
                for at in range(CT):
                    nc.tensor.matmul(
                        ps[:],
                        G_sb[at][:, bt * P:(bt + 1) * P],
                        wq_sb[:, at * C:(at + 1) * C],
                        start=(at == 0),
                        stop=(at == CT - 1),
                    )
                t = small.tile([P, C], BF16, tag=f"T2T_{bt}")
                if bt % 2 == 0:
                    nc.vector.tensor_copy(t[:], ps[:])
                else:
                    nc.scalar.activation(t[:], ps[:], CP)
                T2T_sb.append(t)

            # Diagonal head-pair tiles of D^T = Wk @ T2T ; E^T = exp(scale*D^T)
            # All four D accumulation groups live in one PSUM bank so the
            # exp (ACT) / mask (GpSimd) chain pipelines behind the matmuls.
            # bt is the OUTER loop so the first D matmuls need only T2T_sb[0]
            # and interleave with the remaining T2T casts; each jt group's
            # exp is emitted right after its stop matmul
            dps = [
                gacc.tile([P, P], F32, tag=f"g{jt}", name=f"dps{jt}")
                for jt in range(CT)
            ]
            etmp = small.tile([P, C], F32, tag="etmp")
            for bt in range(CT):
                for jt in range(CT):
                    sl = slice(jt * P, (jt + 1) * P)
                    nc.tensor.matmul(
                        dps[jt][:],
                        wk_sb[:, bt * C + jt * P: bt * C + (jt + 1) * P],
                        T2T_sb[bt][:, sl],
                        start=(bt == 0), stop=(bt == CT - 1),
                    )
                    if bt == CT - 1:
                        nc.scalar.activation(etmp[:, sl], dps[jt][:], EXP,
                                             scale=SCALE)
            ET = []
            for jt in range(CT):
                sl = slice(jt * P, (jt + 1) * P)
                e = small.tile([P, P], BF16, tag=f"ET{jt}", name=f"ET{jt}")
                # zero the cross-head blocks so full-width matmuls (SE,
                # rowsums) see exact per-head separation
                nc.vector.tensor_mul(e[:], etmp[:, sl], dmask[:])
                ET.append(e)

            # per head-pair tile: rowsum matmul, reciprocal, SE = E @ Wv,
            # S = SE/rowsum — interleaved so no PE op queues behind a later
            # tile's mask
            rps = work.tile([P, 2 * CT], F32, tag="wk1", name="rps")
            inv_all = small.tile([P, 2 * CT], F32, tag="inv")
            S_sb = []
            for it in range(CT):
                nc.tensor.matmul(rps[:, 2 * it:2 * it + 2], ET[it][:],
                                 ones2[:], start=True, stop=True)
                inv = inv_all[:, 2 * it:2 * it + 1]
                nc.vector.reciprocal(inv, rps[:, 2 * it:2 * it + 1])
                seps = work.tile([P, C], F32, tag="wk0", name="seps")
                nc.tensor.matmul(
                    seps[:], ET[it][:], wv_sb[:, it * C:(it + 1) * C],
                    start=True, stop=True,
                )
                s_t = small.tile([P, C], BF16, tag=f"S{it}", name=f"S{it}")
                if it % 2 == 0:
                    nc.vector.tensor_scalar_mul(s_t[:], seps[:], inv)
                else:
                    nc.scalar.activation(s_t[:], seps[:], IDENT_FN,
                                         scale=inv)
                S_sb.append(s_t)

            # M^T[c, o] = sum_e S[e][:, c] * WoutT[e][:, o]
            MT_sb = []
            for ct in range(CT):
                ps = work.tile([P, C], F32, tag="wk1", name="mtps")
                for et in range(CT):
                    nc.tensor.matmul(
                        ps[:],
                        S_sb[et][:, ct * P:(ct + 1) * P],
                        wo_sb[:, et * C:(et + 1) * C],
                        start=(et == 0),
                        stop=(et == CT - 1),
                    )
                t = small.tile([P, C], BF16, tag=f"T2T_{ct}", name=f"MT{ct}")
                if ct < 2:
                    if ct % 2 == 0:
                        nc.vector.tensor_copy(t[:], ps[:])
                    else:
                        nc.scalar.activation(t[:], ps[:], CP)
                else:
                    # the late MT tiles gate phase 3's first matmul group:
                    # split their casts across DVE+ACT to halve the latency
                    nc.vector.tensor_copy(t[:, 0:256], ps[:, 0:256])
                    nc.scalar.activation(t[:, 256:C], ps[:, 256:C], CP)
                MT_sb.append(t)

            # ---------- phase 3: out = M @ f_n + bout ----------------------
            # out tiles are staged in ot-pairs and written with one fused DMA
            # per pair (SP and ACT alternate as the issuing engine)
            for ch in range(NCH):
                fnc = fn_ch[ch]
                last = ch == NCH - 1
                for op_ in range(2):
                    stage = outst.tile([P, 2 * 512], F32, tag=f"ost{op_}")
                    for oh in range(2):
                        ot = op_ * 2 + oh
                        ps = gacc.tile([P, 512], F32, tag=f"g{ot}",
                                       name=f"ops{ot}")
                        for ct in range(CT):
                            nc.tensor.matmul(
                                ps[:],
                                MT_sb[ct][:, ot * P:(ot + 1) * P],
                                fnc[:, ct * 512:(ct + 1) * 512],
                                start=(ct == 0),
                                stop=(ct == CT - 1),
                            )
                        if oh == 0:
                            nc.vector.tensor_scalar_add(
                                stage[:, oh * 512:(oh + 1) * 512], ps[:],
                                bout_sb[:, ot:ot + 1],
                            )
                        else:
                            nc.scalar.activation(
                                stage[:, oh * 512:(oh + 1) * 512], ps[:],
                                IDENT_FN, bias=bout_sb[:, ot:ot + 1],
                            )
                        if last:
                            # per-tile DMAs at the end shorten the drain tail
                            eng = nc.sync if oh == 0 else nc.scalar
                            eng.dma_start(
                                out_v[:, ot:ot + 1,
                                      ch * 512:(ch + 1) * 512],
                                stage[:, oh * 512:(oh + 1) * 512]
                                .rearrange("p (o n) -> p o n", o=1),
                            )
                    if not last:
                        dst = out_v[:, 2 * op_:2 * op_ + 2,
                                    ch * 512:(ch + 1) * 512]
                        src = stage[:].rearrange("p (o n) -> p o n", o=2)
                        if op_ == 0:
                            nc.sync.dma_start(dst, src)
                        else:
                            nc.scalar.dma_start(dst, src)

    nc.compile()
    return nc


def _get_nc():
    global _CACHED_NC
    if _CACHED_NC is None:
        _CACHED_NC = _build()
    return _CACHED_NC


def _get_runner():
    """Memoized PJRT runner: jax.jit-compiled once, reused across kernel()
    calls (run_bass_kernel_spmd rebuilds the jit closure every call, which
    forces a ~minute-long recompile)."""
    global _CACHED_RUNNER
    if _CACHED_RUNNER is not None:
        return _CACHED_RUNNER

    import jax
    from jax.sharding import Mesh, PartitionSpec
    from jax.experimental.shard_map import shard_map
    import concourse.mybir as mybir_
    from concourse.bass2jax import (
        _bass_exec_p,
        install_neuronx_cc_hook,
        partition_id_tensor,
    )

    nc = _get_nc()
    install_neuronx_cc_hook()

    partition_name = (
        nc.partition_id_tensor.name if nc.partition_id_tensor else None
    )
    in_names = []
    out_names = []
    out_avals = []
    out_shapes = []
    for alloc in nc.m.functions[0].allocations:
        if not isinstance(alloc, mybir_.MemoryLocationSet):
            continue
        name = alloc.memorylocations[0].name
        if alloc.kind == "ExternalInput":
            if name != partition_name:
                in_names.append(name)
        elif alloc.kind == "ExternalOutput":
            shape = tuple(alloc.tensor_shape)
            dtype = mybir_.dt.np(alloc.dtype)
            out_names.append(name)
            out_avals.append(jax.core.ShapedArray(shape, dtype))
            out_shapes.append((shape, dtype))
    n_params = len(in_names)
    n_outs = len(out_names)
    all_names = tuple(in_names + out_names)
    if partition_name is not None:
        all_names = all_names + (partition_name,)
    donate = tuple(range(n_params, n_params + n_outs))

    def _body(*args):
        operands = list(args)
        if partition_name is not None:
            operands.append(partition_id_tensor())
        outs = _bass_exec_p.bind(
            *operands,
            out_avals=tuple(out_avals),
            in_names=all_names,
            out_names=tuple(out_names),
            lowering_input_output_aliases=(),
            sim_require_finite=True,
            sim_require_nnan=True,
            nc=nc,
        )
        return tuple(outs)

    devices = jax.devices()[:B]
    mesh = Mesh(np.asarray(devices), ("core",))
    sharded = jax.jit(
        shard_map(
            _body,
            mesh=mesh,
            in_specs=(PartitionSpec("core"),) * (n_params + n_outs),
            out_specs=(PartitionSpec("core"),) * n_outs,
            check_rep=False,
        ),
        donate_argnums=donate,
        keep_unused=True,
    )

    def run(in_maps):
        concat_in = [
            np.concatenate([np.asarray(m[k]) for m in in_maps], axis=0)
            for k in in_names
        ]
        concat_zeros = [
            np.zeros((B * s[0], *s[1:]), dt) for (s, dt) in out_shapes
        ]
        out_arrs = sharded(*concat_in, *concat_zeros)
        return [
            {
                k: np.asarray(out_arrs[i]).reshape(B, *out_shapes[i][0])[c]
                for i, k in enumerate(out_names)
            }
            for c in range(B)
        ]

    _CACHED_RUNNER = run
    return run


def kernel(f_m, f_n, Wq, Wkv, Wout, bout, trace=False):
    f_m = np.asarray(f_m, dtype=np.float32)
    f_n = np.asarray(f_n, dtype=np.float32)
    Wq = np.asarray(Wq, dtype=np.float32)
    Wkv = np.asarray(Wkv, dtype=np.float32)
    Wout = np.asarray(Wout, dtype=np.float32)
    bout = np.ascontiguousarray(np.asarray(bout, dtype=np.float32))

    b, c, h, w = f_m.shape
    nc = _get_nc()
    wqt = np.ascontiguousarray(Wq.T.astype(_BF))
    wkt = np.ascontiguousarray(Wkv[:C].T.astype(_BF))
    wv = np.ascontiguousarray(Wkv[C:].astype(_BF))
    woutt = np.ascontiguousarray(Wout.T.astype(_BF))
    fmt_all = np.ascontiguousarray(
        f_m.reshape(b, C, NN).transpose(0, 2, 1).astype(_BF)
    )
    fn_all = f_n.reshape(b, C, NN).astype(_BF)
    in_maps = [
        {
            "fmT": fmt_all[i],
            "fn": fn_all[i],
            "WqT": wqt,
            "WkT": wkt,
            "Wv": wv,
            "WoutT": woutt,
            "bout": bout,
            "ident": _IDENT,
            "dmask": _DMASK,
            "ones2": _ONES2,
        }
        for i in range(b)
    ]
    if trace:
        res = run_bass_kernel_spmd(
            nc, in_maps, core_ids=list(range(B)), trace=True
        )
        kernel.last_results = res
        results = res.results
    else:
        results = _get_runner()(in_maps)
    return np.stack([r["out"].reshape(c, h, w) for r in results])
